# revision 2
# baseline (speedup 1.0000x reference)
"""nn_AttentionModel_6468220748046 kernel.

Self-contained: takes FULL unsharded inputs, returns FULL output [512, 10] f32.
Model: conv1d(stride4) -> BN(inf) -> ReLU -> +PE -> 2x(attn with
distance-weighted softmax + LN) -> LN -> GAP -> linear head.

Execution: data-parallel across the 8 NeuronCores (batch 512 -> 8 x 64)
via jax.pmap; weights (~0.8 MB) are replicated. Falls back to a pure
NumPy implementation if the accelerator path is unavailable.
"""

import math

import numpy as np

SEQ = 179
EMB = 256
HEADS = 8
HDIM = EMB // HEADS
EPS = 1e-5
N_CORES = 8

_WEIGHT_NAMES = (
    "conv_w",
    "conv_b",
    "bn_g",
    "bn_b",
    "bn_mean",
    "bn_var",
    "wq1",
    "wk1",
    "wv1",
    "lnA1_g",
    "lnA1_b",
    "wq2",
    "wk2",
    "wv2",
    "lnA2_g",
    "lnA2_b",
    "ln2_g",
    "ln2_b",
    "out_w",
    "out_b",
)


def _make_pe(d_model=EMB, max_len=SEQ):
    pos = np.arange(max_len, dtype=np.float32)[:, None]
    div = np.exp(
        np.arange(0, d_model, 2, dtype=np.float32) * (-math.log(10000.0) / d_model)
    ).astype(np.float32)
    ang = (pos * div * (d_model / max_len)).astype(np.float32)
    pe = np.stack([np.sin(ang), np.cos(ang)], axis=-1).reshape(max_len, d_model)
    return pe.astype(np.float32)


def _make_sw(n=SEQ):
    idx = np.arange(n, dtype=np.float32)
    return (np.abs(idx[None, :] - idx[:, None]) / n).astype(np.float32)


# ---------------------------------------------------------------------------
# Accelerator path (jax.pmap over 8 NeuronCores)
# ---------------------------------------------------------------------------

_PMAP_CACHE = {}


def _get_pmap_fn():
    if "fn" in _PMAP_CACHE:
        return _PMAP_CACHE["fn"]

    import jax
    import jax.numpy as jnp

    n_dev = len(jax.devices())
    if n_dev < N_CORES:
        raise RuntimeError(f"need {N_CORES} devices, have {n_dev}")

    pe_c = jnp.asarray(_make_pe())
    sw_c = jnp.asarray(_make_sw())

    def ln(x, g, b):
        mu = jnp.mean(x, axis=-1, keepdims=True)
        d = x - mu
        var = jnp.mean(d * d, axis=-1, keepdims=True)
        return d * jax.lax.rsqrt(var + EPS) * g + b

    def attn(x, wq, wk, wv, g, b):
        B, S, E = x.shape
        scale = E ** (-0.5)
        q = (x @ wq.T).reshape(B, S, HEADS, HDIM)
        k = (x @ wk.T).reshape(B, S, HEADS, HDIM)
        v = (x @ wv.T).reshape(B, S, HEADS, HDIM)
        a = jnp.einsum("bshd,bthd->bhst", q, k) * scale
        a = a * sw_c
        a = jax.nn.softmax(a, axis=-1)
        o = jnp.einsum("bhst,bthd->bshd", a, v).reshape(B, S, E)
        return ln(o, g, b)

    def fwd(
        x,
        conv_w,
        conv_b,
        bn_g,
        bn_b,
        bn_mean,
        bn_var,
        wq1,
        wk1,
        wv1,
        lnA1_g,
        lnA1_b,
        wq2,
        wk2,
        wv2,
        lnA2_g,
        lnA2_b,
        ln2_g,
        ln2_b,
        out_w,
        out_b,
    ):
        xs = x[:, 0, :]  # [B, 720]
        # conv1d(stride 4, K=8, VALID) as patch matmul: window t, tap k -> 4t+k
        patches = jnp.stack(
            [xs[:, k : k + 4 * SEQ - 3 : 4] for k in range(8)], axis=-1
        )  # [B, 179, 8]
        wc = conv_w[:, 0, :].T  # [8, EMB]
        h = patches @ wc + conv_b  # [B, 179, EMB]
        inv = jax.lax.rsqrt(bn_var + EPS)
        h = (h - bn_mean) * (bn_g * inv) + bn_b
        h = jnp.maximum(h, 0.0)
        x1 = h + pe_c
        a1 = attn(x1, wq1, wk1, wv1, lnA1_g, lnA1_b)
        x2 = a1 + pe_c
        a2 = attn(x2, wq2, wk2, wv2, lnA2_g, lnA2_b)
        a2 = ln(a2, ln2_g, ln2_b)
        pooled = jnp.mean(a2, axis=1)  # [B, EMB]
        return pooled @ out_w.T + out_b

    fn = jax.pmap(fwd, in_axes=(0,) + (None,) * len(_WEIGHT_NAMES))
    _PMAP_CACHE["fn"] = fn
    return fn


def _kernel_device(x, weights):
    fn = _get_pmap_fn()
    B = x.shape[0]
    per = B // N_CORES
    xs = np.ascontiguousarray(x.reshape(N_CORES, per, 1, 720))
    out = fn(xs, *[weights[n] for n in _WEIGHT_NAMES])
    out = np.asarray(out, dtype=np.float32).reshape(B, 10)
    return out


# ---------------------------------------------------------------------------
# NumPy fallback
# ---------------------------------------------------------------------------


def _np_layernorm(x, g, b):
    mu = np.mean(x, axis=-1, keepdims=True, dtype=np.float32)
    d = x - mu
    var = np.mean(d * d, axis=-1, keepdims=True, dtype=np.float32)
    return (d / np.sqrt(var + np.float32(EPS))) * g + b


def _np_attention(x, wq, wk, wv, g, b, sw):
    B, S, E = x.shape
    scale = np.float32(E ** (-0.5))
    q = (x @ wq.T).reshape(B, S, HEADS, HDIM).transpose(0, 2, 1, 3)
    k = (x @ wk.T).reshape(B, S, HEADS, HDIM).transpose(0, 2, 1, 3)
    v = (x @ wv.T).reshape(B, S, HEADS, HDIM).transpose(0, 2, 1, 3)
    attn = (q @ k.transpose(0, 1, 3, 2)).astype(np.float32) * scale
    attn *= sw[None, None]
    attn -= attn.max(axis=-1, keepdims=True)
    np.exp(attn, out=attn)
    attn /= attn.sum(axis=-1, keepdims=True, dtype=np.float32)
    out = (attn @ v).transpose(0, 2, 1, 3).reshape(B, S, E)
    return _np_layernorm(out, g, b)


def _kernel_numpy(x, w):
    pe = _make_pe()
    sw = _make_sw()
    B = x.shape[0]
    xs = x[:, 0, :]
    sv = np.lib.stride_tricks.sliding_window_view(xs, 8, axis=1)
    patches = sv[:, ::4, :]
    wc = np.ascontiguousarray(w["conv_w"][:, 0, :].T)
    inv = (1.0 / np.sqrt(w["bn_var"] + np.float32(EPS))).astype(np.float32)
    a = (w["bn_g"] * inv).astype(np.float32)
    h = (patches.reshape(-1, 8) @ wc).reshape(B, SEQ, EMB)
    h = (h + w["conv_b"] - w["bn_mean"]) * a + w["bn_b"]
    np.maximum(h, 0.0, out=h)
    x1 = h + pe[None]
    att = _np_attention(x1, w["wq1"], w["wk1"], w["wv1"], w["lnA1_g"], w["lnA1_b"], sw)
    x2 = att + pe[None]
    att = _np_attention(x2, w["wq2"], w["wk2"], w["wv2"], w["lnA2_g"], w["lnA2_b"], sw)
    att = _np_layernorm(att, w["ln2_g"], w["ln2_b"])
    pooled = att.mean(axis=1, dtype=np.float32)
    return (pooled @ w["out_w"].T + w["out_b"]).astype(np.float32)


def kernel(x, **weights):
    x = np.asarray(x, dtype=np.float32)
    w = {n: np.asarray(weights[n], dtype=np.float32) for n in _WEIGHT_NAMES}
    try:
        return _kernel_device(x, w)
    except Exception:
        return _kernel_numpy(x, w)


# revision 4
# speedup vs baseline: 72.0708x; 72.0708x over previous
"""nn_AttentionModel_6468220748046 kernel.

Self-contained: takes FULL unsharded inputs, returns FULL output [512, 10] f32.
Model: conv1d(stride4) -> BN(inf) -> ReLU -> +PE -> 2x(attn with
distance-weighted softmax + LN) -> LN -> GAP -> linear head.

Execution: data-parallel across the 8 NeuronCores (batch 512 -> 8 x 64)
via jax.pmap; weights (~0.8 MB) are replicated. Falls back to a pure
NumPy implementation if the accelerator path is unavailable.
"""

import math

import numpy as np

SEQ = 179
EMB = 256
HEADS = 8
HDIM = EMB // HEADS
EPS = 1e-5
N_CORES = 8

_WEIGHT_NAMES = (
    "conv_w",
    "conv_b",
    "bn_g",
    "bn_b",
    "bn_mean",
    "bn_var",
    "wq1",
    "wk1",
    "wv1",
    "lnA1_g",
    "lnA1_b",
    "wq2",
    "wk2",
    "wv2",
    "lnA2_g",
    "lnA2_b",
    "ln2_g",
    "ln2_b",
    "out_w",
    "out_b",
)


def _make_pe(d_model=EMB, max_len=SEQ):
    pos = np.arange(max_len, dtype=np.float32)[:, None]
    div = np.exp(
        np.arange(0, d_model, 2, dtype=np.float32) * (-math.log(10000.0) / d_model)
    ).astype(np.float32)
    ang = (pos * div * (d_model / max_len)).astype(np.float32)
    pe = np.stack([np.sin(ang), np.cos(ang)], axis=-1).reshape(max_len, d_model)
    return pe.astype(np.float32)


def _make_sw(n=SEQ):
    idx = np.arange(n, dtype=np.float32)
    return (np.abs(idx[None, :] - idx[:, None]) / n).astype(np.float32)


# ---------------------------------------------------------------------------
# Accelerator path (jax.pmap over 8 NeuronCores)
# ---------------------------------------------------------------------------

_PMAP_CACHE = {}


def _get_pmap_fn():
    if "fn" in _PMAP_CACHE:
        return _PMAP_CACHE["fn"]

    import jax
    import jax.numpy as jnp  # noqa: F401

    n_dev = len(jax.devices())
    if n_dev < N_CORES:
        raise RuntimeError(f"need {N_CORES} devices, have {n_dev}")

    pe_c = jnp.asarray(_make_pe())
    sw_c = jnp.asarray(_make_sw())

    def ln(x, g, b):
        mu = jnp.mean(x, axis=-1, keepdims=True)
        d = x - mu
        var = jnp.mean(d * d, axis=-1, keepdims=True)
        return d * jax.lax.rsqrt(var + EPS) * g + b

    def attn(x, wq, wk, wv, g, b):
        B, S, E = x.shape
        scale = E ** (-0.5)
        q = (x @ wq.T).reshape(B, S, HEADS, HDIM)
        k = (x @ wk.T).reshape(B, S, HEADS, HDIM)
        v = (x @ wv.T).reshape(B, S, HEADS, HDIM)
        a = jnp.einsum("bshd,bthd->bhst", q, k) * scale
        a = a * sw_c
        a = jax.nn.softmax(a, axis=-1)
        o = jnp.einsum("bhst,bthd->bshd", a, v).reshape(B, S, E)
        return ln(o, g, b)

    def fwd(
        x,
        conv_w,
        conv_b,
        bn_g,
        bn_b,
        bn_mean,
        bn_var,
        wq1,
        wk1,
        wv1,
        lnA1_g,
        lnA1_b,
        wq2,
        wk2,
        wv2,
        lnA2_g,
        lnA2_b,
        ln2_g,
        ln2_b,
        out_w,
        out_b,
    ):
        xs = x[:, 0, :]  # [B, 720]
        # conv1d(stride 4, K=8, VALID) as patch matmul: window t, tap k -> 4t+k
        patches = jnp.stack(
            [xs[:, k : k + 4 * SEQ - 3 : 4] for k in range(8)], axis=-1
        )  # [B, 179, 8]
        wc = conv_w[:, 0, :].T  # [8, EMB]
        h = patches @ wc + conv_b  # [B, 179, EMB]
        inv = jax.lax.rsqrt(bn_var + EPS)
        h = (h - bn_mean) * (bn_g * inv) + bn_b
        h = jnp.maximum(h, 0.0)
        x1 = h + pe_c
        a1 = attn(x1, wq1, wk1, wv1, lnA1_g, lnA1_b)
        x2 = a1 + pe_c
        a2 = attn(x2, wq2, wk2, wv2, lnA2_g, lnA2_b)
        a2 = ln(a2, ln2_g, ln2_b)
        pooled = jnp.mean(a2, axis=1)  # [B, EMB]
        return pooled @ out_w.T + out_b

    # All args mapped over axis 0: x is sharded, weights are pre-replicated
    # device arrays cached across calls (saves re-broadcasting ~0.8 MB x 8
    # through the device proxy on every invocation).
    fn = jax.pmap(fwd, in_axes=0)
    _PMAP_CACHE["fn"] = fn
    return fn


def _weights_fingerprint(w):
    h = 0
    for n in _WEIGHT_NAMES:
        h ^= hash((n, w[n].tobytes()))
    return h


def _get_replicated_weights(w):
    fp = _weights_fingerprint(w)
    if _PMAP_CACHE.get("wfp") != fp:
        import jax

        devs = jax.devices()[:N_CORES]
        _PMAP_CACHE["ws"] = [
            jax.device_put_replicated(w[n], devs) for n in _WEIGHT_NAMES
        ]
        _PMAP_CACHE["wfp"] = fp
    return _PMAP_CACHE["ws"]


def _kernel_device(x, weights):
    fn = _get_pmap_fn()
    ws = _get_replicated_weights(weights)
    B = x.shape[0]
    per = B // N_CORES
    xs = np.ascontiguousarray(x.reshape(N_CORES, per, 1, 720))
    out = fn(xs, *ws)
    out = np.asarray(out, dtype=np.float32).reshape(B, 10)
    return out


# ---------------------------------------------------------------------------
# NumPy fallback
# ---------------------------------------------------------------------------


def _np_layernorm(x, g, b):
    mu = np.mean(x, axis=-1, keepdims=True, dtype=np.float32)
    d = x - mu
    var = np.mean(d * d, axis=-1, keepdims=True, dtype=np.float32)
    return (d / np.sqrt(var + np.float32(EPS))) * g + b


def _np_attention(x, wq, wk, wv, g, b, sw):
    B, S, E = x.shape
    scale = np.float32(E ** (-0.5))
    q = (x @ wq.T).reshape(B, S, HEADS, HDIM).transpose(0, 2, 1, 3)
    k = (x @ wk.T).reshape(B, S, HEADS, HDIM).transpose(0, 2, 1, 3)
    v = (x @ wv.T).reshape(B, S, HEADS, HDIM).transpose(0, 2, 1, 3)
    attn = (q @ k.transpose(0, 1, 3, 2)).astype(np.float32) * scale
    attn *= sw[None, None]
    attn -= attn.max(axis=-1, keepdims=True)
    np.exp(attn, out=attn)
    attn /= attn.sum(axis=-1, keepdims=True, dtype=np.float32)
    out = (attn @ v).transpose(0, 2, 1, 3).reshape(B, S, E)
    return _np_layernorm(out, g, b)


def _kernel_numpy(x, w):
    pe = _make_pe()
    sw = _make_sw()
    B = x.shape[0]
    xs = x[:, 0, :]
    sv = np.lib.stride_tricks.sliding_window_view(xs, 8, axis=1)
    patches = sv[:, ::4, :]
    wc = np.ascontiguousarray(w["conv_w"][:, 0, :].T)
    inv = (1.0 / np.sqrt(w["bn_var"] + np.float32(EPS))).astype(np.float32)
    a = (w["bn_g"] * inv).astype(np.float32)
    h = (patches.reshape(-1, 8) @ wc).reshape(B, SEQ, EMB)
    h = (h + w["conv_b"] - w["bn_mean"]) * a + w["bn_b"]
    np.maximum(h, 0.0, out=h)
    x1 = h + pe[None]
    att = _np_attention(x1, w["wq1"], w["wk1"], w["wv1"], w["lnA1_g"], w["lnA1_b"], sw)
    x2 = att + pe[None]
    att = _np_attention(x2, w["wq2"], w["wk2"], w["wv2"], w["lnA2_g"], w["lnA2_b"], sw)
    att = _np_layernorm(att, w["ln2_g"], w["ln2_b"])
    pooled = att.mean(axis=1, dtype=np.float32)
    return (pooled @ w["out_w"].T + w["out_b"]).astype(np.float32)


def kernel(x, **weights):
    x = np.asarray(x, dtype=np.float32)
    w = {n: np.asarray(weights[n], dtype=np.float32) for n in _WEIGHT_NAMES}
    try:
        return _kernel_device(x, w)
    except Exception:
        return _kernel_numpy(x, w)


# revision 5
# speedup vs baseline: 109.4412x; 1.5185x over previous
"""nn_AttentionModel_6468220748046 kernel.

Self-contained: takes FULL unsharded inputs, returns FULL output [512, 10] f32.
Model: conv1d(stride4) -> BN(inf) -> ReLU -> +PE -> 2x(attn with
distance-weighted softmax + LN) -> LN -> GAP -> linear head.

Execution: data-parallel across the 8 NeuronCores (batch 512 -> 8 x 64)
via jax.pmap; weights (~0.8 MB) are replicated. Falls back to a pure
NumPy implementation if the accelerator path is unavailable.
"""

import math

import numpy as np

SEQ = 179
EMB = 256
HEADS = 8
HDIM = EMB // HEADS
EPS = 1e-5
N_CORES = 8

_WEIGHT_NAMES = (
    "conv_w",
    "conv_b",
    "bn_g",
    "bn_b",
    "bn_mean",
    "bn_var",
    "wq1",
    "wk1",
    "wv1",
    "lnA1_g",
    "lnA1_b",
    "wq2",
    "wk2",
    "wv2",
    "lnA2_g",
    "lnA2_b",
    "ln2_g",
    "ln2_b",
    "out_w",
    "out_b",
)


def _make_pe(d_model=EMB, max_len=SEQ):
    pos = np.arange(max_len, dtype=np.float32)[:, None]
    div = np.exp(
        np.arange(0, d_model, 2, dtype=np.float32) * (-math.log(10000.0) / d_model)
    ).astype(np.float32)
    ang = (pos * div * (d_model / max_len)).astype(np.float32)
    pe = np.stack([np.sin(ang), np.cos(ang)], axis=-1).reshape(max_len, d_model)
    return pe.astype(np.float32)


def _make_sw(n=SEQ):
    idx = np.arange(n, dtype=np.float32)
    return (np.abs(idx[None, :] - idx[:, None]) / n).astype(np.float32)


# ---------------------------------------------------------------------------
# Accelerator path (jax.pmap over 8 NeuronCores)
# ---------------------------------------------------------------------------

_PMAP_CACHE = {}


def _get_pmap_fn():
    if "fn" in _PMAP_CACHE:
        return _PMAP_CACHE["fn"]

    import jax
    import jax.numpy as jnp  # noqa: F401

    n_dev = len(jax.devices())
    if n_dev < N_CORES:
        raise RuntimeError(f"need {N_CORES} devices, have {n_dev}")

    pe_c = jnp.asarray(_make_pe())
    sw_c = jnp.asarray(_make_sw())

    def ln(x, g, b):
        mu = jnp.mean(x, axis=-1, keepdims=True)
        d = x - mu
        var = jnp.mean(d * d, axis=-1, keepdims=True)
        return d * jax.lax.rsqrt(var + EPS) * g + b

    bf16 = jnp.bfloat16
    f32 = jnp.float32

    def attn(x, wq, wk, wv, g, b):
        # matmuls in bf16 (2x PE rate, half the HBM traffic on the
        # [B,H,S,S] attention tensor); softmax stats + LN stay f32.
        B, S, E = x.shape
        scale = E ** (-0.5)
        x16 = x.astype(bf16)
        q = (x16 @ wq.T.astype(bf16)).reshape(B, S, HEADS, HDIM)
        k = (x16 @ wk.T.astype(bf16)).reshape(B, S, HEADS, HDIM)
        v = (x16 @ wv.T.astype(bf16)).reshape(B, S, HEADS, HDIM)
        a = jnp.einsum("bshd,bthd->bhst", q, k, preferred_element_type=f32)
        a = a * (sw_c * scale)
        a = jax.nn.softmax(a, axis=-1)
        o = jnp.einsum(
            "bhst,bthd->bshd", a.astype(bf16), v, preferred_element_type=f32
        ).reshape(B, S, E)
        return ln(o, g, b)

    def fwd(
        x,
        conv_w,
        conv_b,
        bn_g,
        bn_b,
        bn_mean,
        bn_var,
        wq1,
        wk1,
        wv1,
        lnA1_g,
        lnA1_b,
        wq2,
        wk2,
        wv2,
        lnA2_g,
        lnA2_b,
        ln2_g,
        ln2_b,
        out_w,
        out_b,
    ):
        xs = x[:, 0, :]  # [B, 720]
        # conv1d(stride 4, K=8, VALID) as patch matmul: window t, tap k -> 4t+k
        patches = jnp.stack(
            [xs[:, k : k + 4 * SEQ - 3 : 4] for k in range(8)], axis=-1
        )  # [B, 179, 8]
        wc = conv_w[:, 0, :].T  # [8, EMB]
        h = patches @ wc + conv_b  # [B, 179, EMB]
        inv = jax.lax.rsqrt(bn_var + EPS)
        h = (h - bn_mean) * (bn_g * inv) + bn_b
        h = jnp.maximum(h, 0.0)
        x1 = h + pe_c
        a1 = attn(x1, wq1, wk1, wv1, lnA1_g, lnA1_b)
        x2 = a1 + pe_c
        a2 = attn(x2, wq2, wk2, wv2, lnA2_g, lnA2_b)
        a2 = ln(a2, ln2_g, ln2_b)
        pooled = jnp.mean(a2, axis=1)  # [B, EMB]
        return pooled @ out_w.T + out_b

    # All args mapped over axis 0: x is sharded, weights are pre-replicated
    # device arrays cached across calls (saves re-broadcasting ~0.8 MB x 8
    # through the device proxy on every invocation).
    fn = jax.pmap(fwd, in_axes=0)
    _PMAP_CACHE["fn"] = fn
    return fn


def _weights_fingerprint(w):
    h = 0
    for n in _WEIGHT_NAMES:
        h ^= hash((n, w[n].tobytes()))
    return h


def _get_replicated_weights(w):
    fp = _weights_fingerprint(w)
    if _PMAP_CACHE.get("wfp") != fp:
        import jax

        devs = jax.devices()[:N_CORES]
        _PMAP_CACHE["ws"] = [
            jax.device_put_replicated(w[n], devs) for n in _WEIGHT_NAMES
        ]
        _PMAP_CACHE["wfp"] = fp
    return _PMAP_CACHE["ws"]


def _kernel_device(x, weights):
    fn = _get_pmap_fn()
    ws = _get_replicated_weights(weights)
    B = x.shape[0]
    per = B // N_CORES
    xs = np.ascontiguousarray(x.reshape(N_CORES, per, 1, 720))
    out = fn(xs, *ws)
    out = np.asarray(out, dtype=np.float32).reshape(B, 10)
    return out


# ---------------------------------------------------------------------------
# NumPy fallback
# ---------------------------------------------------------------------------


def _np_layernorm(x, g, b):
    mu = np.mean(x, axis=-1, keepdims=True, dtype=np.float32)
    d = x - mu
    var = np.mean(d * d, axis=-1, keepdims=True, dtype=np.float32)
    return (d / np.sqrt(var + np.float32(EPS))) * g + b


def _np_attention(x, wq, wk, wv, g, b, sw):
    B, S, E = x.shape
    scale = np.float32(E ** (-0.5))
    q = (x @ wq.T).reshape(B, S, HEADS, HDIM).transpose(0, 2, 1, 3)
    k = (x @ wk.T).reshape(B, S, HEADS, HDIM).transpose(0, 2, 1, 3)
    v = (x @ wv.T).reshape(B, S, HEADS, HDIM).transpose(0, 2, 1, 3)
    attn = (q @ k.transpose(0, 1, 3, 2)).astype(np.float32) * scale
    attn *= sw[None, None]
    attn -= attn.max(axis=-1, keepdims=True)
    np.exp(attn, out=attn)
    attn /= attn.sum(axis=-1, keepdims=True, dtype=np.float32)
    out = (attn @ v).transpose(0, 2, 1, 3).reshape(B, S, E)
    return _np_layernorm(out, g, b)


def _kernel_numpy(x, w):
    pe = _make_pe()
    sw = _make_sw()
    B = x.shape[0]
    xs = x[:, 0, :]
    sv = np.lib.stride_tricks.sliding_window_view(xs, 8, axis=1)
    patches = sv[:, ::4, :]
    wc = np.ascontiguousarray(w["conv_w"][:, 0, :].T)
    inv = (1.0 / np.sqrt(w["bn_var"] + np.float32(EPS))).astype(np.float32)
    a = (w["bn_g"] * inv).astype(np.float32)
    h = (patches.reshape(-1, 8) @ wc).reshape(B, SEQ, EMB)
    h = (h + w["conv_b"] - w["bn_mean"]) * a + w["bn_b"]
    np.maximum(h, 0.0, out=h)
    x1 = h + pe[None]
    att = _np_attention(x1, w["wq1"], w["wk1"], w["wv1"], w["lnA1_g"], w["lnA1_b"], sw)
    x2 = att + pe[None]
    att = _np_attention(x2, w["wq2"], w["wk2"], w["wv2"], w["lnA2_g"], w["lnA2_b"], sw)
    att = _np_layernorm(att, w["ln2_g"], w["ln2_b"])
    pooled = att.mean(axis=1, dtype=np.float32)
    return (pooled @ w["out_w"].T + w["out_b"]).astype(np.float32)


def kernel(x, **weights):
    x = np.asarray(x, dtype=np.float32)
    w = {n: np.asarray(weights[n], dtype=np.float32) for n in _WEIGHT_NAMES}
    try:
        return _kernel_device(x, w)
    except Exception:
        return _kernel_numpy(x, w)
